# revision 22
# baseline (speedup 1.0000x reference)
"""KeepTopK kernel for Trainium2.

out[i, j] = x[i, j] if x[i, j] is among the top-8 of row i else 1e6.

Pure data parallel on 8 cores (32768 rows each). Per [128, 2048] block
(1024 rows, 8 rows per partition):
  load   : 2MB SWDGE loads (two blocks per load; DMASW sem lanes keep
           load completions decoupled from store completions on DMAHW).
           First pair loads via HWDGE 1MB to dodge the GpSimd Q7
           descriptor-generator cold-start.
  DVE    : per 256-wide row segment s: v8 = max8(x_seg); then
           P_seg = (x_seg < t_s) * BETA  via scalar_tensor_tensor with
           t_s = v8[:, s*8+7] (8th largest, [128,1] scalar operand) and
           a BETA-filled constant tile as in1.  P is 0 at top-8
           positions and BETA elsewhere.
  POOL   : o = P + x  (whole block; exact x at top-8, BETA+x ~ BETA
           elsewhere, relative error ~1e-6)
  store  : 1MB from SP (qSPDynamicHW).

Tie semantics: an element equal to the 8th-largest is always kept, so
rows whose 8th/9th order statistics are bit-equal keep >8 values
(~5 rows in 262144, rel Frobenius error ~3e-4 vs 2e-2 tolerance).
"""
import numpy as np
from contextlib import ExitStack

import concourse.bass as bass
import concourse.mybir as mybir
import concourse.tile as tile
from concourse.bass_utils import run_bass_kernel_spmd

N, E, K = 262144, 256, 8
BETA = 1000000.0
NCORES = 8
ROWS_PER_CORE = N // NCORES          # 32768
ROWS_PER_PART = 8                    # rows packed per SBUF partition
BLOCK_FREE = ROWS_PER_PART * E       # 2048
ROWS_PER_BLOCK = 128 * ROWS_PER_PART  # 1024
NBLOCKS = ROWS_PER_CORE // ROWS_PER_BLOCK  # 32

MAX_WAITS = 1


def split_sync_waits(nc, max_waits=MAX_WAITS):
    """walrus codegen rejects instructions with more than one embedded sync
    wait; hoist extras onto same-engine NoOps placed immediately before."""
    spill_id = 0
    for f in nc.m.functions:
        for bb in f.blocks:
            insts = list(bb.instructions)
            new_insts = []
            changed = False
            for inst in insts:
                si = inst.sync_info
                waits = list(si.on_wait) if si and si.on_wait else []
                if len(waits) > max_waits:
                    extra = waits[:-max_waits]
                    si.on_wait = waits[-max_waits:]
                    for j in range(0, len(extra), max_waits):
                        nop = mybir.InstNoOp(
                            name=f"waitspill-{spill_id}", ins=[], outs=[])
                        spill_id += 1
                        nop.engine = inst.engine
                        nop.sync_info = type(si)(
                            on_wait=extra[j:j + max_waits], on_update=[])
                        new_insts.append(nop)
                    changed = True
                new_insts.append(inst)
            if changed:
                bb.instructions = new_insts


def build():
    nc = bass.Bass("TRN2", target_bir_lowering=False, debug=False)
    x = nc.dram_tensor("x", [ROWS_PER_CORE, E], mybir.dt.float32,
                       kind="ExternalInput")
    out = nc.dram_tensor("out", [ROWS_PER_CORE, E], mybir.dt.float32,
                         kind="ExternalOutput")
    xap = x.ap()
    oap = out.ap()
    f32 = mybir.dt.float32
    with tile.TileContext(nc) as tc:
        with ExitStack() as ctx:
            xpool = ctx.enter_context(tc.tile_pool(name="x", bufs=3))
            ppool = ctx.enter_context(tc.tile_pool(name="pp", bufs=4))
            opool = ctx.enter_context(tc.tile_pool(name="o", bufs=5))
            vpool = ctx.enter_context(tc.tile_pool(name="v8", bufs=6))
            cpool = ctx.enter_context(tc.tile_pool(name="cbeta", bufs=1))
            beta = cpool.tile([128, E], f32)
            nc.vector.memset(beta[:], BETA)
            xt2 = None
            for b in range(NBLOCKS):
                r0 = b * ROWS_PER_BLOCK
                dst = oap[r0:r0 + ROWS_PER_BLOCK, :].rearrange(
                    "(p r) e -> p (r e)", p=128)
                src = xap[r0:r0 + ROWS_PER_BLOCK, :].rearrange(
                    "(p r) e -> p (r e)", p=128)
                if b < 2:
                    # HWDGE for the first pair: SWDGE (Q7) has a long
                    # cold-start and there are no stores yet, so sharing
                    # the DMAHW sem lanes is harmless here.
                    if b % 2 == 0:
                        xt2 = xpool.tile([128, 2 * BLOCK_FREE], f32,
                                         tag="x2")
                    nc.sync.dma_start(
                        xt2[:, (b % 2) * BLOCK_FREE:
                            (b % 2 + 1) * BLOCK_FREE], src)
                elif b % 2 == 0:
                    # partition p holds rows [r0+8p : r0+8p+8] of block b
                    # (c=0) then the same rows of block b+1 (c=1)
                    src2 = xap[r0:r0 + 2 * ROWS_PER_BLOCK, :].rearrange(
                        "(c p r) e -> p c (r e)", c=2, p=128)
                    xt2 = xpool.tile([128, 2 * BLOCK_FREE], f32, tag="x2")
                    nc.gpsimd.dma_start(xt2[:], src2)
                boff = (b % 2) * BLOCK_FREE
                v8 = vpool.tile([128, 8 * ROWS_PER_PART], f32)
                for s in range(ROWS_PER_PART):
                    seg = slice(boff + s * E, boff + (s + 1) * E)
                    nc.vector.max(v8[:, s * 8:(s + 1) * 8], xt2[:, seg])
                pt = ppool.tile([128, BLOCK_FREE], f32)
                for s in range(ROWS_PER_PART):
                    seg = slice(boff + s * E, boff + (s + 1) * E)
                    nc.vector.scalar_tensor_tensor(
                        pt[:, s * E:(s + 1) * E], xt2[:, seg],
                        v8[:, s * 8 + 7:s * 8 + 8], beta[:],
                        op0=mybir.AluOpType.is_lt,
                        op1=mybir.AluOpType.mult)
                ot = opool.tile([128, BLOCK_FREE], f32)
                nc.gpsimd.tensor_tensor(
                    ot[:], pt[:], xt2[:, boff:boff + BLOCK_FREE],
                    op=mybir.AluOpType.add)
                nc.sync.dma_start(dst, ot[:])
    split_sync_waits(nc)
    return nc


_nc_cache = None


def _get_nc():
    global _nc_cache
    if _nc_cache is None:
        _nc_cache = build()
    return _nc_cache


def kernel(x: np.ndarray, _trace: bool = False, **_trace_kwargs):
    x = np.ascontiguousarray(np.asarray(x, dtype=np.float32))
    assert x.shape == (N, E), x.shape
    nc = _get_nc()
    in_maps = [
        {"x": x[c * ROWS_PER_CORE:(c + 1) * ROWS_PER_CORE]}
        for c in range(NCORES)
    ]
    res = run_bass_kernel_spmd(nc, in_maps, core_ids=list(range(NCORES)),
                               trace=_trace, **_trace_kwargs)
    out = np.concatenate([res.results[c]["out"] for c in range(NCORES)],
                         axis=0)
    if _trace:
        return out, res
    return out


# revision 23
# speedup vs baseline: 1.2987x; 1.2987x over previous
"""KeepTopK kernel for Trainium2.

out[i, j] = x[i, j] if x[i, j] is among the top-8 of row i else 1e6.

Strategy (pure data parallel, 8 cores, 32768 rows each):
  per [128, 2048] block (1024 rows, 8 rows per partition):
    load  : 1MB SWDGE loads (nc.gpsimd) — SWDGE completions use the
            DMASW semaphore lanes, disjoint from the stores' DMAHW
            lanes, so a slow store receipt can never false-block a
            load consumer.  Block 0 loads as 2x 512KB via HWDGE
            (nc.sync): faster first-byte than the cold Q7 SWDGE path,
            and half-granular so DVE starts ~3us earlier.
    DVE   : per 256-wide row segment: v8 = max8(x_seg) then
            y = match_replace(x, v8, BETA)   (top-8 positions -> BETA)
    ACT   : z = -y + BETA        per half-block (0 at top-8, else ~BETA)
    POOL  : o = z + x            per half-block (exact x at top-8)
    store : per half-block 512KB from SP (qSPDynamicHW).  The last
            block runs its ACT/POOL/store stages per quarter-block to
            shorten the drain chain.
match_replace replaces exactly one occurrence per top-8 element in index
order, matching jax.lax.top_k tie semantics bitwise.
"""
import numpy as np
from contextlib import ExitStack

import concourse.bass as bass
import concourse.mybir as mybir
import concourse.tile as tile
from concourse.bass_utils import run_bass_kernel_spmd

N, E, K = 262144, 256, 8
BETA = 1000000.0
NCORES = 8
ROWS_PER_CORE = N // NCORES          # 32768
ROWS_PER_PART = 8                    # rows packed per SBUF partition
BLOCK_FREE = ROWS_PER_PART * E       # 2048
ROWS_PER_BLOCK = 128 * ROWS_PER_PART  # 1024
NBLOCKS = ROWS_PER_CORE // ROWS_PER_BLOCK  # 32
HALF = BLOCK_FREE // 2               # 1024

MAX_WAITS = 1


def split_sync_waits(nc, max_waits=MAX_WAITS):
    """walrus codegen rejects instructions with more than one embedded sync
    wait; hoist extras onto same-engine NoOps placed immediately before."""
    spill_id = 0
    for f in nc.m.functions:
        for bb in f.blocks:
            insts = list(bb.instructions)
            new_insts = []
            changed = False
            for inst in insts:
                si = inst.sync_info
                waits = list(si.on_wait) if si and si.on_wait else []
                if len(waits) > max_waits:
                    extra = waits[:-max_waits]
                    si.on_wait = waits[-max_waits:]
                    for j in range(0, len(extra), max_waits):
                        nop = mybir.InstNoOp(
                            name=f"waitspill-{spill_id}", ins=[], outs=[])
                        spill_id += 1
                        nop.engine = inst.engine
                        nop.sync_info = type(si)(
                            on_wait=extra[j:j + max_waits], on_update=[])
                        new_insts.append(nop)
                    changed = True
                new_insts.append(inst)
            if changed:
                bb.instructions = new_insts


def build():
    nc = bass.Bass("TRN2", target_bir_lowering=False, debug=False)
    x = nc.dram_tensor("x", [ROWS_PER_CORE, E], mybir.dt.float32,
                       kind="ExternalInput")
    out = nc.dram_tensor("out", [ROWS_PER_CORE, E], mybir.dt.float32,
                         kind="ExternalOutput")
    xap = x.ap()
    oap = out.ap()
    f32 = mybir.dt.float32
    with tile.TileContext(nc) as tc:
        with ExitStack() as ctx:
            xpool = ctx.enter_context(tc.tile_pool(name="x", bufs=8))
            ypool = ctx.enter_context(tc.tile_pool(name="y", bufs=8))
            zpool = ctx.enter_context(tc.tile_pool(name="z", bufs=8))
            opool = ctx.enter_context(tc.tile_pool(name="o", bufs=10))
            vpool = ctx.enter_context(tc.tile_pool(name="v8", bufs=8))
            for b in range(NBLOCKS):
                r0 = b * ROWS_PER_BLOCK
                src = xap[r0:r0 + ROWS_PER_BLOCK, :].rearrange(
                    "(p r) e -> p (r e)", p=128)
                dst = oap[r0:r0 + ROWS_PER_BLOCK, :].rearrange(
                    "(p r) e -> p (r e)", p=128)
                xt = xpool.tile([128, BLOCK_FREE], f32)
                if b == 0:
                    # HWDGE half-loads: no Q7 cold start, DVE can begin
                    # on the first half while the second streams in.
                    nc.sync.dma_start(xt[:, :HALF], src[:, :HALF])
                    nc.sync.dma_start(xt[:, HALF:], src[:, HALF:])
                else:
                    nc.gpsimd.dma_start(xt[:], src)
                # epilogue granularity: quarters for the last block to
                # shorten the drain chain, halves otherwise
                nparts = 4 if b == NBLOCKS - 1 else 2
                pw = BLOCK_FREE // nparts        # columns per part
                segs = pw // E                   # segments per part
                for h in range(nparts):
                    h0 = h * pw
                    yt = ypool.tile([128, pw], f32, tag="y")
                    v8 = vpool.tile([128, 8 * segs], f32, tag="v8")
                    for s in range(segs):
                        seg = slice(h0 + s * E, h0 + (s + 1) * E)
                        v = v8[:, s * 8:(s + 1) * 8]
                        nc.vector.max(v, xt[:, seg])
                        nc.vector.match_replace(
                            yt[:, s * E:(s + 1) * E], v, xt[:, seg], BETA)
                    zt = zpool.tile([128, pw], f32, tag="z")
                    nc.scalar.activation(zt[:], yt[:],
                                         mybir.ActivationFunctionType.Copy,
                                         bias=BETA, scale=-1.0)
                    ot = opool.tile([128, pw], f32, tag="o")
                    nc.gpsimd.tensor_tensor(ot[:], zt[:], xt[:, h0:h0 + pw],
                                            op=mybir.AluOpType.add)
                    nc.sync.dma_start(dst[:, h0:h0 + pw], ot[:])
    split_sync_waits(nc)
    return nc


_nc_cache = None


def _get_nc():
    global _nc_cache
    if _nc_cache is None:
        _nc_cache = build()
    return _nc_cache


def kernel(x: np.ndarray, _trace: bool = False, **_trace_kwargs):
    x = np.ascontiguousarray(np.asarray(x, dtype=np.float32))
    assert x.shape == (N, E), x.shape
    nc = _get_nc()
    in_maps = [
        {"x": x[c * ROWS_PER_CORE:(c + 1) * ROWS_PER_CORE]}
        for c in range(NCORES)
    ]
    res = run_bass_kernel_spmd(nc, in_maps, core_ids=list(range(NCORES)),
                               trace=_trace, **_trace_kwargs)
    out = np.concatenate([res.results[c]["out"] for c in range(NCORES)],
                         axis=0)
    if _trace:
        return out, res
    return out
